# revision 1
# baseline (speedup 1.0000x reference)
"""Trainium2 Bass kernel for nn_BatchRankingLoss (n=8192, 8 NeuronCores).

Math: reference computes sum over pairs i<j of relu(-(p_j-p_i)*sign(l_j-l_i) + 2).
The sum runs over UNORDERED pairs and is invariant to re-indexing, so we sort by
labels on the host: with q = preds[argsort(labels)], the loss becomes
    sum_{u<v} relu(2 + q_u - q_v)
(plus an exact O(#ties) host correction for tied labels, where sign()=0).

Device strategy (SPMD, 8 cores, one shared program). 64 row-tiles of 128 rows;
core k owns tiles {k+16m, 15-k+16m}, presented as 8 fixed-width "slots" of
[16,14,12,10,8,6,4,2] 512-col chunks (window starts at the diagonal block;
unused tail columns zero-padded). Three engines are saturated in parallel:

- PE route (46 chunks, slot-proportional, incl. every diagonal chunk):
  K=16 bf16 matmul per 512-col chunk -> t = q_u + (2 - q_v) in f32 PSUM (rhs
  packed into 8 partition-pair "streams", zero lhsT lanes select the stream).
  Diagonal chunks get a second [128,128] matmul adding -1e9 on the lower
  triangle. PSUM groups are reduced by either:
    ACT: activation(Relu, accum_out) -> sum relu(t)
    DVE: tensor_reduce(add, abs) -> sum |t|, combined with the analytic linear
         term sum(t) (affine in q_u, per-core inputs) via relu = (t + |t|)/2.
- ACT-direct route (26 chunks): a broadcast tile QB[128, 13312] holds
  bf16(2 - q_v) replicated across partitions; activation(Relu, bias=q_u,
  accum_out) computes sum_v relu(2 - q_v + q_u) in ONE ACT pass (no PE, no
  separate reduce). Padded columns hold -1000 so relu kills them.

Each core outputs a [128,1] partial; host sums 8x128 partials + tie correction.
"""

import numpy as np

N = 8192
NBLK = 64
SLOT_CHUNKS = [16, 14, 12, 10, 8, 6, 4, 2]    # 512-col chunks per slot
PE_CHUNKS = [10, 9, 8, 7, 5, 4, 2, 1]         # chunks on the PE route per slot
ALT_CHUNKS = [c - p for c, p in zip(SLOT_CHUNKS, PE_CHUNKS)]   # ACT-direct
N_PE = sum(PE_CHUNKS)                          # 46
N_ALT = sum(ALT_CHUNKS)                        # 26
STREAM_CAP = 6                                 # PE chunks per stream (6*512)
QB_COLS = N_ALT * 512                          # 13312
PENALTY = -1.0e9
PAD_VAL = -1000.0

# ---------------------------------------------------------------------------
# Stream packing for the PE route
# ---------------------------------------------------------------------------

def _pack_streams():
    chunk_map = {}
    variants = []
    vmap = {}
    stream = 0
    pos = 0
    for s, nch in enumerate(PE_CHUNKS):
        for c in range(nch):
            if pos == STREAM_CAP:
                stream += 1
                pos = 0
            chunk_map[(s, c)] = (stream, pos)
            if (s, stream) not in vmap:
                vmap[(s, stream)] = len(variants)
                variants.append((s, stream))
            pos += 1
    assert stream <= 7, (stream, pos)
    return chunk_map, variants, vmap

CHUNK_MAP, VARIANTS, VMAP = _pack_streams()
NVAR = len(VARIANTS)

# ALT segment offsets in QB (per slot), in columns
ALT_OFFS = []
_o = 0
for _c in ALT_CHUNKS:
    ALT_OFFS.append(_o)
    _o += _c * 512
assert _o == QB_COLS

# ---------------------------------------------------------------------------
# Schedule: PE-route reduce groups + engine assignment
# ---------------------------------------------------------------------------

def make_schedule():
    """PE-route groups: (slot, chunk0, nchunks, is_diag, engine)."""
    groups = []
    for s, nch in enumerate(PE_CHUNKS):
        c = 0
        while c < nch:
            if c == 0 and s < 4:
                g = 1            # narrow diag group: shifts reduce work to DVE
            else:
                g = min(2, nch - c)
            groups.append([s, c, g, c == 0])
            c += g
    # diag groups forced to ACT; others balance DVE-heavy (ACT also runs the
    # ACT-direct route, so give DVE everything it can take)
    act_cost = sum(ALT_CHUNKS) * 512 * 0.8333 + 8 * 370.0   # ACT-direct load
    dve_cost = 0.0
    sched = []
    for s, c0, g, diag in groups:
        w = g * 512
        ca = w * 0.8333 + 290.0
        cd = w * 1.0417 + 170.0
        if diag:
            eng = "A"
        else:
            eng = "A" if act_cost + ca <= dve_cost + cd else "D"
        if eng == "A":
            act_cost += ca
        else:
            dve_cost += cd
        sched.append((s, c0, g, diag, eng))
    return sched

SCHEDULE = make_schedule()

# ---------------------------------------------------------------------------
# Device program
# ---------------------------------------------------------------------------

_CACHE = {}

def build_program():
    import concourse.bacc as bacc
    import concourse.mybir as mybir
    from concourse.tile import TileContext

    F32 = mybir.dt.float32
    BF16 = mybir.dt.bfloat16
    AX = mybir.AxisListType
    OP = mybir.AluOpType
    AF = mybir.ActivationFunctionType

    nA = sum(1 for g in SCHEDULE if g[4] == "A") + 8   # + 8 ACT-direct groups
    nD = sum(1 for g in SCHEDULE if g[4] == "D")

    nc = bacc.Bacc(trn_type="TRN2")
    rhs_d = nc.dram_tensor("rhs", [16, STREAM_CAP * 512], BF16, kind="ExternalInput")
    lhs_d = nc.dram_tensor("lhs", [16, NVAR * 128], BF16, kind="ExternalInput")
    tri_d = nc.dram_tensor("tri", [128, 128], BF16, kind="ExternalInput")
    pen_d = nc.dram_tensor("pen", [128, 128], BF16, kind="ExternalInput")
    qb_d = nc.dram_tensor("qb", [128, QB_COLS], BF16, kind="ExternalInput")
    qcol_d = nc.dram_tensor("qcol", [128, 8], F32, kind="ExternalInput")
    lin_d = nc.dram_tensor("linab", [128, 16], F32, kind="ExternalInput")
    out_d = nc.dram_tensor("out", [128, 1], F32, kind="ExternalOutput")

    with TileContext(nc) as tc:
        with tc.tile_pool(name="consts", bufs=1) as cpool, \
             tc.tile_pool(name="scr", bufs=2) as spool, \
             tc.tile_pool(name="ps", bufs=4, space="PSUM") as psp:
            RHS = cpool.tile([16, STREAM_CAP * 512], BF16)
            LHS = cpool.tile([16, NVAR * 128], BF16)
            TRI = cpool.tile([128, 128], BF16)
            PEN = cpool.tile([128, 128], BF16)
            QB = cpool.tile([128, QB_COLS], BF16)
            QCOL = cpool.tile([128, 8], F32)
            LIN = cpool.tile([128, 16], F32)
            ACCA = cpool.tile([128, nA], F32)
            ACCD = cpool.tile([128, max(nD, 1)], F32)
            ACCL = cpool.tile([128, 8], F32)
            R = cpool.tile([128, 4], F32)
            OUT = cpool.tile([128, 1], F32)

            nc.sync.dma_start(out=RHS[:], in_=rhs_d[:])
            nc.sync.dma_start(out=LHS[:], in_=lhs_d[:])
            nc.sync.dma_start(out=TRI[:], in_=tri_d[:])
            nc.sync.dma_start(out=PEN[:], in_=pen_d[:])
            nc.sync.dma_start(out=QCOL[:], in_=qcol_d[:])
            nc.sync.dma_start(out=LIN[:], in_=lin_d[:])
            # QB streamed per-slot so ACT-direct groups start early
            for s in range(8):
                w = ALT_CHUNKS[s] * 512
                if w:
                    nc.sync.dma_start(out=QB[:, ALT_OFFS[s]:ALT_OFFS[s] + w],
                                      in_=qb_d[:, ALT_OFFS[s]:ALT_OFFS[s] + w])

            # dep-free PE warmup while input DMAs are in flight
            DW = cpool.tile([128, 512], BF16)
            nc.gpsimd.memset(DW[:], 0.0)
            WPS = psp.tile([128, 1024], F32, tag="ps")
            for _ in range(4):
                nc.tensor.matmul(WPS[:, 0:512], DW[0:16, 0:128], DW[0:16, 0:512],
                                 start=True, stop=True)

            ia = 0
            id_ = 0
            alt_done = [False] * 8
            for gi, (s, c0, g, diag, eng) in enumerate(SCHEDULE):
                w = g * 512
                PS = psp.tile([128, 1024], F32, tag="ps")
                for b in range(g):
                    st, pos = CHUNK_MAP[(s, c0 + b)]
                    v = VMAP[(s, st)]
                    nc.tensor.matmul(PS[:, b * 512:(b + 1) * 512],
                                     LHS[:, v * 128:(v + 1) * 128],
                                     RHS[:, pos * 512:(pos + 1) * 512],
                                     start=True, stop=not (diag and b == 0))
                if diag:
                    nc.tensor.matmul(PS[:, 0:128], TRI[:], PEN[:],
                                     start=False, stop=True)
                if eng == "A":
                    SCR = spool.tile([128, 1024], F32, tag="scr")
                    nc.scalar.activation(out=SCR[:, :w], in_=PS[:, :w], func=AF.Relu,
                                         bias=0.0, scale=1.0,
                                         accum_out=ACCA[:, ia:ia + 1])
                    ia += 1
                else:
                    nc.vector.tensor_reduce(out=ACCD[:, id_:id_ + 1], in_=PS[:, :w],
                                            axis=AX.X, op=OP.add,
                                            apply_absolute_value=True)
                    id_ += 1
                # interleave ACT-direct groups after this slot's PE groups
                if not alt_done[s]:
                    last_of_slot = all(SCHEDULE[j][0] != s for j in
                                       range(gi + 1, len(SCHEDULE)))
                    if last_of_slot and ALT_CHUNKS[s] > 0:
                        wq = ALT_CHUNKS[s] * 512
                        SCR2 = spool.tile([128, 4096], F32, tag="scr2")
                        nc.scalar.activation(out=SCR2[:, :wq],
                                             in_=QB[:, ALT_OFFS[s]:ALT_OFFS[s] + wq],
                                             func=AF.Relu,
                                             bias=QCOL[:, s:s + 1], scale=1.0,
                                             accum_out=ACCA[:, ia:ia + 1])
                        ia += 1
                        alt_done[s] = True

            # linear terms: accL[:, s] = A_s * q_u + B_s
            for s in range(8):
                nc.vector.tensor_scalar(ACCL[:, s:s + 1], QCOL[:, s:s + 1],
                                        LIN[:, 2 * s:2 * s + 1],
                                        LIN[:, 2 * s + 1:2 * s + 2],
                                        OP.mult, OP.add)

            # combine: out = sum(ACCA) + 0.5*(sum(ACCD) + sum(ACCL))
            nc.vector.tensor_reduce(out=R[:, 0:1], in_=ACCA[:], axis=AX.X, op=OP.add)
            nc.vector.tensor_reduce(out=R[:, 1:2], in_=ACCD[:], axis=AX.X, op=OP.add)
            nc.vector.tensor_reduce(out=R[:, 2:3], in_=ACCL[:], axis=AX.X, op=OP.add)
            nc.vector.tensor_tensor(out=R[:, 1:2], in0=R[:, 1:2], in1=R[:, 2:3],
                                    op=OP.add)
            nc.vector.tensor_scalar(R[:, 1:2], R[:, 1:2], 0.5, None, OP.mult)
            nc.vector.tensor_tensor(out=R[:, 0:1], in0=R[:, 0:1], in1=R[:, 1:2],
                                    op=OP.add)
            nc.vector.tensor_copy(out=OUT[:], in_=R[:, 0:1])
            nc.sync.dma_start(out=out_d[:], in_=OUT[:])

    nc.finalize()
    return nc


def get_program():
    if "nc" not in _CACHE:
        _CACHE["nc"] = build_program()
    return _CACHE["nc"]

# ---------------------------------------------------------------------------
# Host side
# ---------------------------------------------------------------------------

def core_tiles(k):
    return sorted([k + 16 * m for m in range(4)] + [15 - k + 16 * m for m in range(4)])


def build_inputs(q):
    """Per-core in_maps for label-sorted preds q (np.float32 [8192])."""
    import ml_dtypes
    BF = ml_dtypes.bfloat16
    q = q.astype(np.float32)
    qb16 = q.astype(BF)
    rhs1_full = (2.0 - q).astype(np.float32).astype(BF)
    tri = np.triu(np.ones((128, 128), np.float32)).astype(BF)
    pen = np.zeros((128, 128), np.float32)
    pen[np.arange(128), np.arange(128)] = PENALTY
    pen = pen.astype(BF)

    in_maps = []
    for k in range(8):
        tiles = core_tiles(k)
        rhs = np.zeros((16, STREAM_CAP * 512), BF)
        lhs = np.zeros((16, NVAR * 128), BF)
        qbt = np.full((128, QB_COLS), PAD_VAL, np.float32).astype(BF)
        qcol = np.zeros((128, 8), np.float32)
        lin = np.zeros((128, 16), np.float32)
        for s, t in enumerate(tiles):
            real = (NBLK - t) * 128
            qcol[:, s] = qb16[t * 128:(t + 1) * 128].astype(np.float32)
            # PE-route chunks
            for c in range(PE_CHUNKS[s]):
                st, pos = CHUNK_MAP[(s, c)]
                lo = c * 512
                take = min(max(real - lo, 0), 512)
                if take > 0:
                    rhs[2 * st, pos * 512: pos * 512 + take] = np.float32(1.0)
                    rhs[2 * st + 1, pos * 512: pos * 512 + take] = \
                        rhs1_full[t * 128 + lo: t * 128 + lo + take]
                v = VMAP[(s, st)]
                lhs[2 * st, v * 128:(v + 1) * 128] = qb16[t * 128:(t + 1) * 128]
                lhs[2 * st + 1, v * 128:(v + 1) * 128] = np.float32(1.0)
            # ACT-direct chunks (tail of the window)
            for a in range(ALT_CHUNKS[s]):
                lo = (PE_CHUNKS[s] + a) * 512
                take = min(max(real - lo, 0), 512)
                col0 = ALT_OFFS[s] + a * 512
                if take > 0:
                    qbt[:, col0:col0 + take] = \
                        rhs1_full[t * 128 + lo: t * 128 + lo + take][None, :]
            # linear terms over this slot's DVE groups
            A = 0.0
            B = 0.0
            for (gs, c0, g, diag, eng) in SCHEDULE:
                if gs != s or eng != "D":
                    continue
                for b in range(g):
                    st, pos = CHUNK_MAP[(s, c0 + b)]
                    A += rhs[2 * st, pos * 512:(pos + 1) * 512].astype(np.float64).sum()
                    B += rhs[2 * st + 1, pos * 512:(pos + 1) * 512].astype(np.float64).sum()
            lin[:, 2 * s] = np.float32(A)
            lin[:, 2 * s + 1] = np.float32(B)
        in_maps.append({"rhs": rhs, "lhs": lhs, "tri": tri, "pen": pen,
                        "qb": qbt, "qcol": qcol, "linab": lin})
    return in_maps


def emulate(in_maps):
    """Numpy emulation of the device program (for offline validation)."""
    total = 0.0
    for k in range(8):
        m = in_maps[k]
        rhs = m["rhs"].astype(np.float32)
        lhs = m["lhs"].astype(np.float32)
        tri = m["tri"].astype(np.float32)
        pen = m["pen"].astype(np.float32)
        qb = m["qb"].astype(np.float32)
        qcol = m["qcol"]
        lin = m["linab"]
        accA = 0.0
        accD = 0.0
        accL = 0.0
        for (s, c0, g, diag, eng) in SCHEDULE:
            ps = np.zeros((128, g * 512), np.float64)
            for b in range(g):
                st, pos = CHUNK_MAP[(s, c0 + b)]
                v = VMAP[(s, st)]
                L = lhs[:, v * 128:(v + 1) * 128]
                Rr = rhs[:, pos * 512:(pos + 1) * 512]
                ps[:, b * 512:(b + 1) * 512] = L.T @ Rr
            if diag:
                ps[:, 0:128] += tri.T @ pen
            if eng == "A":
                accA += np.maximum(ps, 0).sum()
            else:
                accD += np.abs(ps).sum()
        for s in range(8):
            wq = ALT_CHUNKS[s] * 512
            if wq:
                t = qb[:, ALT_OFFS[s]:ALT_OFFS[s] + wq] + qcol[:, s][:, None]
                accA += np.maximum(t, 0).sum()
            accL += (lin[0, 2 * s] * qcol[:, s] + lin[0, 2 * s + 1]).sum()
        total += accA + 0.5 * (accD + accL)
    return total


def tie_correction(labels, q, order):
    ls = labels[order]
    corr = 0.0
    i = 0
    n = len(ls)
    while i < n:
        j = i + 1
        while j < n and ls[j] == ls[i]:
            j += 1
        if j - i > 1:
            for u in range(i, j):
                for v in range(u + 1, j):
                    corr += 2.0 - max(0.0, 2.0 + float(q[u]) - float(q[v]))
        i = j
    return corr


def run(inputs, trace=False):
    from concourse.bass_utils import run_bass_kernel_spmd

    preds = np.asarray(inputs["preds"], dtype=np.float32)
    labels = np.asarray(inputs["labels"], dtype=np.float32)
    order = np.argsort(labels, kind="stable")
    q = preds[order]

    nc = get_program()
    in_maps = build_inputs(q)
    res = run_bass_kernel_spmd(nc, in_maps, core_ids=list(range(8)), trace=trace)
    total = 0.0
    for c in range(8):
        total += res.results[c]["out"].astype(np.float64).sum()
    total += tie_correction(labels, q, order)
    return np.float32(total), res


def kernel(**inputs):
    out, _ = run(inputs, trace=False)
    return out



# revision 12
# speedup vs baseline: 1.1146x; 1.1146x over previous
"""Trainium2 Bass kernel for nn_BatchRankingLoss (n=8192, 8 NeuronCores).

Math: reference computes sum over pairs i<j of relu(-(p_j-p_i)*sign(l_j-l_i) + 2).
Sorting by labels on the host (q = preds[argsort(labels)]) turns this into
    sum_{u<v} relu((2 - q_v) + q_u)
plus an exact O(#ties) host correction for tied labels.

Device strategy (SPMD, one shared program, 8 cores). 64 row-tiles of 128 rows;
core k owns tiles {8s + d_s(k)} (slot s, d alternating by parity => per-core
work exactly balanced). Per slot the off-diagonal window [128(t+1), 8192) is
read from a SHARED broadcast array QB[p, v] = bf16(2 - q_v) (2.3MB instead of
9.4MB of private windows). QB is split into 3 column tiles (with 896-col
overlap margins so every slot/segment statically fits one tile) DMA'd on 3
different queues so compute can chase the DMA tail-first.

Per-core window starts differ, so the consumer (DVE/ACT) instructions live in
an 8-armed tc.Switch on partition_id; each arm bakes that core's static AP
offsets. Segment widths are the per-slot maxima; the <=896-col dead tail per
segment lands in QB's PAD columns (relu kills it). The PE stream stays
outside the switch: each arm writes the same S-ring tiles, and PE matmuls
with an all-ones stationary [128,128] sum S over partitions into one PSUM
accumulator (colsum; every output row equals the route total).

Routes (measured): DVE tensor_scalar relu bf16 4x = 0.286 ns/col feeding PE
matmul sum = 0.42 ns/col (pipelined, PE-bound); ACT activation(Relu, bias,
accum_out) = 0.87 ns/col. Diagonal tiles are host-prebuilt bias-folded masked
tiles (QBD) on the PE route. Final: DVE reduces the PSUM colsums and the ACT
accums; host sums cores + tie correction.
"""

import numpy as np

N = 8192
NBLK = 64
PAD_VAL = -1000.0
QB_W = 9088          # 8192 + 896 static-width spill pad

# QB column tiles (base, width). Overlap >= 896 so any 896-uncertain window
# slice fits statically inside one tile.
TILES = [(0, 4224), (3328, 3776), (6208, 2880)]

TMIN = [8 * s for s in range(8)]                  # slot s min tile index
WS = [(63 - 8 * s) * 128 for s in range(8)]       # static off-diag widths

# measured engine rates (ns per 128-elem column) and fixed per-instr costs
R_PE, R_DVE, R_ACT = 0.4167, 0.287, 0.870
F_PE, F_DVE, F_ACT = 10.0, 80.0, 560.0


def tile_of(s, c, w):
    """Tile index j such that [off+c, off+c+w) fits tile j for every core."""
    lo = (TMIN[s] + 1) * 128 + c
    hi = (TMIN[s] + 8) * 128 + c + w
    for j, (b, tw) in enumerate(TILES):
        if lo >= b and hi <= b + tw:
            return j
    return None


def slot_tile_spans(s):
    """Split slot s's window [0, WS[s]) into per-tile spans, deepest first."""
    spans = []  # (tile_j, c, w)
    c = 0
    while c < WS[s]:
        best = None
        for j in (0, 1, 2):
            b, tw = TILES[j]
            lo_min = (TMIN[s] + 1) * 128 + c
            hi_cap = b + tw - (TMIN[s] + 8) * 128   # max c+w for this tile
            if lo_min >= b and hi_cap > c:
                w = min(WS[s], hi_cap) - c
                if best is None or w > best[1]:
                    best = (j, w)
        assert best is not None, (s, c)
        spans.append((best[0], c, best[1]))
        c += best[1]
    return spans


def make_schedule():
    """Assign (slot, tile, c, w) segments to engines, balancing clocks.

    Returns list of ('PE'|'ACT', s, j, c, w) in emission order (chase order:
    tile 2 segments first, then 1, then 0). The diag QBD rides the PE route
    up front.
    """
    segs = []
    for s in range(8):
        for (j, c, w) in slot_tile_spans(s):
            segs.append([s, j, c, w])
    segs.sort(key=lambda x: (-x[1], -x[0]))

    pe_t = 1024 * R_PE            # QBD already on PE route
    act_t = 0.0
    out = []
    for s, j, c, w in segs:
        t_pe = pe_t + w * R_PE + F_PE
        t_act = act_t + w * R_ACT + F_ACT
        if t_act < t_pe and w >= 1024:
            act_t = t_act
            out.append(("ACT", s, j, c, w))
        else:
            pe_t = t_pe
            out.append(("PE", s, j, c, w))
    return out


SCHEDULE = make_schedule()
N_PE_SEGS = sum(1 for e in SCHEDULE if e[0] == "PE")
N_ACT_SEGS = sum(1 for e in SCHEDULE if e[0] == "ACT")

_CACHE = {}


def core_tiles(k):
    """Slot s tile for core k: 8s + (k if s odd else 7-k); exact balance."""
    return [8 * s + (k if s % 2 == 1 else 7 - k) for s in range(8)]


def build_program():
    import concourse.bacc as bacc
    import concourse.mybir as mybir
    from concourse.tile import TileContext

    F32 = mybir.dt.float32
    BF16 = mybir.dt.bfloat16
    AX = mybir.AxisListType
    OP = mybir.AluOpType
    AF = mybir.ActivationFunctionType
    ET = mybir.EngineType

    nc = bacc.Bacc(trn_type="TRN2")
    qb_t0_d = nc.dram_tensor("qb_t0", [128, TILES[0][1]], BF16, kind="ExternalInput")
    qb_t1_d = nc.dram_tensor("qb_t1", [128, TILES[1][1]], BF16, kind="ExternalInput")
    qb_t2_d = nc.dram_tensor("qb_t2", [128, TILES[2][1]], BF16, kind="ExternalInput")
    qbd_d = nc.dram_tensor("qbd", [128, 1024], BF16, kind="ExternalInput")
    qcol_d = nc.dram_tensor("qcol", [128, 8], F32, kind="ExternalInput")
    outp_d = nc.dram_tensor("outp", [128, 1], F32, kind="ExternalOutput")
    outa_d = nc.dram_tensor("outa", [128, 1], F32, kind="ExternalOutput")

    with TileContext(nc) as tc:
        with tc.tile_pool(name="consts", bufs=1) as cpool, \
             tc.tile_pool(name="ps", bufs=1, space="PSUM") as psp, \
             tc.tile_pool(name="psw", bufs=2, space="PSUM") as psw:
            QBT = [cpool.tile([128, TILES[j][1]], BF16, name=f"QBT{j}")
                   for j in range(3)]
            QBD = cpool.tile([128, 1024], BF16)
            QCOL = cpool.tile([128, 8], F32)
            ONES = cpool.tile([128, 128], BF16)
            DW = cpool.tile([128, 512], BF16)
            ACCA = cpool.tile([128, max(N_ACT_SEGS, 1)], F32)
            RESP = cpool.tile([128, 1], F32)
            RESA = cpool.tile([128, 1], F32)

            # --- DMAs: metadata first; QB tiles on 3 queues, deepest first
            nc.sync.dma_start(out=QCOL[:], in_=qcol_d[:])
            nc.sync.dma_start(out=QBD[:], in_=qbd_d[:])
            nc.sync.dma_start(out=QBT[2][:], in_=qb_t2_d[:])
            nc.scalar.dma_start(out=QBT[1][:], in_=qb_t1_d[:])
            nc.gpsimd.dma_start(out=QBT[0][:], in_=qb_t0_d[:])

            # --- constants + PE warmup + ACT table preload (during DMA) ---
            nc.gpsimd.memset(ONES[:], 1.0)
            nc.gpsimd.memset(DW[:], 0.0)
            for _ in range(10):
                WPS = psw.tile([128, 512], F32, tag="warm")
                nc.tensor.matmul(WPS[:], ONES[:], DW[:], start=True, stop=True)
            AWU = cpool.tile([128, 1], F32)
            nc.scalar.activation(out=AWU[:], in_=DW[:, 0:1], func=AF.Relu,
                                 bias=0.0, scale=1.0)

            core_id = tc.nc.partition_id(engines=[ET.DVE, ET.Activation])

            PS = psp.tile([128, 512], F32, tag="acc")

            # --- QBD (diag tiles): DVE relu -> S, PE sums (chain start) ---
            SD = cpool.tile([128, 1024], BF16)
            # One exactly-sized S buffer per PE-route segment (written inside
            # arms, consumed by the static PE stream outside the switch; no
            # buffer reuse may cross the switch reconvergence barrier).
            pe_ws = [e[4] for e in SCHEDULE if e[0] == "PE"]
            S_tiles = [cpool.tile([128, pe_ws[i]], BF16, name=f"S{i}")
                       for i in range(N_PE_SEGS)]
            act_ws = [e[4] for e in SCHEDULE if e[0] == "ACT"]
            A_tiles = [cpool.tile([128, act_ws[i]], BF16, name=f"A{i}")
                       for i in range(N_ACT_SEGS)]

            nc.vector.tensor_scalar(SD[:, 0:1024], QBD[:], 0.0, 0.0,
                                    OP.add, OP.max)

            # --- consumer instructions, switched per core ---
            for k in tc.Switch(core_id, 8):
                tiles = core_tiles(k)
                ip = 0
                ia = 0
                for kind, s, j, c, w in SCHEDULE:
                    off = (tiles[s] + 1) * 128 + c - TILES[j][0]
                    ap = QBT[j][:, off:off + w]
                    if kind == "ACT":
                        nc.scalar.activation(out=A_tiles[ia][:, 0:w], in_=ap,
                                             func=AF.Relu,
                                             bias=QCOL[:, s:s + 1], scale=1.0,
                                             accum_out=ACCA[:, ia:ia + 1])
                        ia += 1
                    else:
                        nc.vector.tensor_scalar(S_tiles[ip][:, 0:w], ap,
                                                QCOL[:, s:s + 1],
                                                0.0, OP.add, OP.max)
                        ip += 1

            # --- static PE stream: QBD then the S ring, one PSUM chain ---
            nc.tensor.matmul(PS[:], ONES[:], SD[:, 0:512], start=True, stop=False)
            nc.tensor.matmul(PS[:], ONES[:], SD[:, 512:1024], start=False, stop=False)
            pe_widths = [e[4] for e in SCHEDULE if e[0] == "PE"]
            for i, w in enumerate(pe_widths):
                c2 = 0
                while c2 < w:
                    wp = min(512, w - c2)
                    nc.tensor.matmul(PS[:, 0:wp], ONES[:],
                                     S_tiles[i][:, c2:c2 + wp],
                                     start=False,
                                     stop=(i == len(pe_widths) - 1) and
                                          (c2 + wp >= w))
                    c2 += wp

            # --- final combine ---
            nc.vector.tensor_reduce(out=RESP[:], in_=PS[:], axis=AX.X, op=OP.add)
            nc.vector.tensor_reduce(out=RESA[:], in_=ACCA[:], axis=AX.X, op=OP.add)
            nc.sync.dma_start(out=outp_d[:], in_=RESP[:])
            nc.sync.dma_start(out=outa_d[:], in_=RESA[:])

    nc.finalize()
    return nc


def get_program():
    if "nc" not in _CACHE:
        _CACHE["nc"] = build_program()
    return _CACHE["nc"]


# ---------------------------------------------------------------------------
# Host side
# ---------------------------------------------------------------------------

def build_inputs(q):
    import ml_dtypes
    BF = ml_dtypes.bfloat16
    q = q.astype(np.float32)
    rhs = np.full(QB_W, np.float32(PAD_VAL), np.float32)
    rhs[0:N] = 2.0 - q
    rhs16 = rhs.astype(BF)
    qb_rows = {j: np.broadcast_to(rhs16[TILES[j][0]:TILES[j][0] + TILES[j][1]],
                                  (128, TILES[j][1])).copy()
               for j in range(3)}

    in_maps = []
    for k in range(8):
        tiles = core_tiles(k)
        qcol = np.zeros((128, 8), np.float32)
        qbd = np.full((128, 1024), PAD_VAL, np.float32)
        for s, t in enumerate(tiles):
            qcol[:, s] = q[t * 128:(t + 1) * 128]
            blk = q[t * 128:(t + 1) * 128]
            m = (2.0 - blk[None, :]) + blk[:, None]   # [p, c] = 2 - q_v + q_u
            tri = np.triu(np.ones((128, 128), bool), k=1)
            qbd[:, s * 128:(s + 1) * 128] = np.where(tri, m, PAD_VAL)
        in_maps.append({
            "qb_t0": qb_rows[0], "qb_t1": qb_rows[1], "qb_t2": qb_rows[2],
            "qbd": qbd.astype(BF), "qcol": qcol,
        })
    return in_maps


def tie_correction(labels, q, order):
    ls = labels[order]
    corr = 0.0
    i = 0
    n = len(ls)
    while i < n:
        j = i + 1
        while j < n and ls[j] == ls[i]:
            j += 1
        if j - i > 1:
            for u in range(i, j):
                for v in range(u + 1, j):
                    corr += 2.0 - max(0.0, 2.0 + float(q[u]) - float(q[v]))
        i = j
    return corr


def emulate(in_maps):
    """Numpy emulation of the device program for offline validation."""
    import ml_dtypes
    BF = ml_dtypes.bfloat16
    total = 0.0
    for k in range(8):
        m = in_maps[k]
        qbt = [m["qb_t0"].astype(np.float32), m["qb_t1"].astype(np.float32),
               m["qb_t2"].astype(np.float32)]
        qbd = m["qbd"].astype(np.float32)
        qcol = m["qcol"]
        tiles = core_tiles(k)
        core = np.maximum(qbd, 0.0).astype(BF).astype(np.float64).sum()
        acc = 0.0
        for kind, s, j, c, w in SCHEDULE:
            off = (tiles[s] + 1) * 128 + c - TILES[j][0]
            sl = qbt[j][:, off: off + w]
            t = sl + qcol[:, s][:, None]
            r = np.maximum(t, 0.0)
            if kind == "PE":
                core += r.astype(BF).astype(np.float64).sum()
            else:
                acc += r.astype(np.float64).sum()
        total += core + acc
    return total


def run(inputs, trace=False):
    from concourse.bass_utils import run_bass_kernel_spmd

    preds = np.asarray(inputs["preds"], dtype=np.float32)
    labels = np.asarray(inputs["labels"], dtype=np.float32)
    order = np.argsort(labels, kind="stable")
    q = preds[order]

    nc = get_program()
    in_maps = build_inputs(q)
    res = run_bass_kernel_spmd(nc, in_maps, core_ids=list(range(8)), trace=trace)
    total = 0.0
    for c in range(8):
        total += float(res.results[c]["outp"][0, 0])
        total += res.results[c]["outa"].astype(np.float64).sum()
    total += tie_correction(labels, q, order)
    return np.float32(total), res


def kernel(**inputs):
    out, _ = run(inputs, trace=False)
    return out


# revision 14
# speedup vs baseline: 1.5683x; 1.4070x over previous
"""Trainium2 Bass kernel for nn_BatchRankingLoss (n=8192, 8 NeuronCores).

Math: reference computes sum over pairs i<j of relu(-(p_j-p_i)*sign(l_j-l_i) + 2).
Sorting by labels on the host (q = preds[argsort(labels)]) turns this into
    sum_{u<v} relu((2 - q_v) + q_u)
plus an exact O(#ties) host correction for tied labels.

Device strategy (SPMD, one shared program, 8 cores). 64 row-tiles of 128 rows;
core k owns tiles {8s + d_s(k)} (slot s, d alternating by parity => per-core
work exactly balanced). Per slot the off-diagonal window [128(t+1), 8192) is
read from a SHARED broadcast array QB[p, v] = bf16(2 - q_v) (2.3MB instead of
9.4MB of private windows), split into 3 column tiles spread over 3 DMA queues
(HWDGE scalar, SWDGE gpsimd, HWDGE sync) so the deep tile lands first and
compute chases the DMA.

Per-core window starts differ, so consumer (DVE/ACT) instructions live in an
8-armed tc.Switch on partition_id; each arm bakes that core's static AP
offsets and splits segments at QB-tile boundaries (arms may differ in
instruction count; the S-tile structure is shared). Three routes, rates
measured on HW:
 - PE route: DVE tensor_scalar relu bf16 4x (~0.3 ns/col) into per-segment S
   buffers; PE matmuls with an all-ones stationary [128,128] sum S over
   partitions into one PSUM accumulator (~0.5 ns/col warm, chunk cadence).
 - ACT route: activation(Relu, bias, accum_out), ~1.2 ns/col.
 - STT route: DVE scalar_tensor_tensor (in0+bias) max ZERO, accum_out=sum,
   1x (~1.1 ns/col) soaking leftover DVE capacity.
Diagonal tiles are host-prebuilt bias-folded masked tiles (QBD) on the PE
route. Final: DVE reduces PSUM colsums (every partition = route total) and
ACT/STT accums into one [128,2] output; host sums cores + tie correction.
"""

import numpy as np

N = 8192
PAD_VAL = -1000.0
QB_W = 9088          # 8192 + 896 static-width spill pad

# QB column tiles (base, width): exact partition, no overlap.
TILES = [(0, 3008), (3008, 3200), (6208, 2880)]

TMIN = [8 * s for s in range(8)]
WS = [(63 - 8 * s) * 128 for s in range(8)]       # static off-diag widths

# per-slot route blocks (cols): [0,z) STT | [z, z+y) ACT | [z+y, W) PE,
# with PE taking the deep end (earliest-landing data).
STT_W = [1280, 1280, 0, 0, 0, 0, 0, 0]
ACT_W = [2304, 2304, 2304, 2304, 0, 0, 0, 0]


def route_blocks():
    """Static (route, slot, c, w) blocks; c relative to window start."""
    blocks = []
    for s in range(8):
        z, y = STT_W[s], ACT_W[s]
        assert z + y < WS[s]
        if z:
            blocks.append(("STT", s, 0, z))
        if y:
            blocks.append(("ACT", s, z, y))
        blocks.append(("PE", s, z + y, WS[s] - z - y))
    return blocks

BLOCKS = route_blocks()
PE_BLOCKS = [b for b in BLOCKS if b[0] == "PE"]
ACT_BLOCKS = [b for b in BLOCKS if b[0] == "ACT"]
STT_BLOCKS = [b for b in BLOCKS if b[0] == "STT"]
# PE emission order: deepest nominal global start first (chase the DMA)
PE_ORDER = sorted(range(len(PE_BLOCKS)),
                  key=lambda i: -((TMIN[PE_BLOCKS[i][1]] + 1) * 128 + PE_BLOCKS[i][2]))

_CACHE = {}


def core_tiles(k):
    """Slot s tile for core k: 8s + (k if s odd else 7-k); exact balance."""
    return [8 * s + (k if s % 2 == 1 else 7 - k) for s in range(8)]


def tile_splits(goff, w):
    """Split global column range [goff, goff+w) at QB tile boundaries.

    Returns [(tile_j, local_off, w_piece), ...]."""
    out = []
    c = goff
    end = goff + w
    while c < end:
        for j, (b, tw) in enumerate(TILES):
            if b <= c < b + tw:
                take = min(end, b + tw) - c
                out.append((j, c - b, take))
                c += take
                break
        else:
            raise AssertionError((goff, w, c))
    return out


def build_program():
    import concourse.bacc as bacc
    import concourse.mybir as mybir
    from concourse.tile import TileContext

    F32 = mybir.dt.float32
    BF16 = mybir.dt.bfloat16
    AX = mybir.AxisListType
    OP = mybir.AluOpType
    AF = mybir.ActivationFunctionType
    ET = mybir.EngineType

    nc = bacc.Bacc(trn_type="TRN2")
    qb_t0_d = nc.dram_tensor("qb_t0", [128, TILES[0][1]], BF16, kind="ExternalInput")
    qb_t1_d = nc.dram_tensor("qb_t1", [128, TILES[1][1]], BF16, kind="ExternalInput")
    qb_t2_d = nc.dram_tensor("qb_t2", [128, TILES[2][1]], BF16, kind="ExternalInput")
    qbd_d = nc.dram_tensor("qbd", [128, 1024], BF16, kind="ExternalInput")
    qcol_d = nc.dram_tensor("qcol", [128, 8], F32, kind="ExternalInput")
    outv_d = nc.dram_tensor("outv", [128, 2], F32, kind="ExternalOutput")

    with TileContext(nc) as tc:
        with tc.tile_pool(name="consts", bufs=1) as cpool, \
             tc.tile_pool(name="ps", bufs=1, space="PSUM") as psp, \
             tc.tile_pool(name="psw", bufs=2, space="PSUM") as psw:
            QBT = [cpool.tile([128, TILES[j][1]], BF16, name=f"QBT{j}")
                   for j in range(3)]
            QBD = cpool.tile([128, 1024], BF16)
            QCOL = cpool.tile([128, 8], F32)
            ONES = cpool.tile([128, 128], BF16)
            DW = cpool.tile([128, 512], BF16)
            ZERO = cpool.tile([128, max(max(STT_W), 1)], BF16)
            ACCA = cpool.tile([128, 16], F32)
            OUTV = cpool.tile([128, 2], F32)
            SD = cpool.tile([128, 1024], BF16)
            S_tiles = [cpool.tile([128, PE_BLOCKS[i][3]], BF16, name=f"S{i}")
                       for i in range(len(PE_BLOCKS))]
            A_tiles = [cpool.tile([128, ACT_BLOCKS[i][3]], BF16, name=f"A{i}")
                       for i in range(len(ACT_BLOCKS))]
            Z_tiles = [cpool.tile([128, STT_BLOCKS[i][3]], BF16, name=f"Z{i}")
                       for i in range(len(STT_BLOCKS))]

            # --- DMAs: fast queues carry the big tiles, deep tile first ---
            nc.scalar.dma_start(out=QBT[2][:], in_=qb_t2_d[:])
            nc.scalar.dma_start(out=QBT[0][:], in_=qb_t0_d[:])
            nc.gpsimd.dma_start(out=QBD[:], in_=qbd_d[:])
            nc.gpsimd.dma_start(out=QBT[1][:], in_=qb_t1_d[:])
            nc.sync.dma_start(out=QCOL[:], in_=qcol_d[:])

            # --- early constants on DVE + partition ids + PE warmup ---
            nc.vector.memset(ONES[:], 1.0)
            nc.vector.memset(DW[:], 0.0)
            nc.vector.memset(ZERO[:], 0.0)
            nc.vector.memset(ACCA[:], 0.0)
            core_id = tc.nc.partition_id(engines=[ET.DVE, ET.Activation])
            for _ in range(9):
                WPS = psw.tile([128, 512], F32, tag="warm")
                nc.tensor.matmul(WPS[:], ONES[:], DW[:], start=True, stop=True)
            AWU = cpool.tile([128, 1], F32)
            nc.scalar.activation(out=AWU[:], in_=DW[:, 0:1], func=AF.Relu,
                                 bias=0.0, scale=1.0)

            PS = psp.tile([128, 512], F32, tag="acc")

            # QBD relu (static): feeds the PSUM chain start
            nc.vector.tensor_scalar(SD[:], QBD[:], 0.0, 0.0, OP.add, OP.max)

            # --- consumer instructions, switched per core ---
            for k in tc.Switch(core_id, 8):
                tiles = core_tiles(k)
                # PE-route relu passes, in global PE_ORDER; split per QB tile
                for i in PE_ORDER:
                    _, s, c, w = PE_BLOCKS[i]
                    goff = (tiles[s] + 1) * 128 + c
                    done = 0
                    for (j, lo, wp) in tile_splits(goff, w):
                        nc.vector.tensor_scalar(S_tiles[i][:, done:done + wp],
                                                QBT[j][:, lo:lo + wp],
                                                QCOL[:, s:s + 1],
                                                0.0, OP.add, OP.max)
                        done += wp
                # ACT route
                for ia, (_, s, c, w) in enumerate(ACT_BLOCKS):
                    goff = (tiles[s] + 1) * 128 + c
                    done = 0
                    for (j, lo, wp) in tile_splits(goff, w):
                        nc.scalar.activation(
                            out=A_tiles[ia][:, done:done + wp],
                            in_=QBT[j][:, lo:lo + wp], func=AF.Relu,
                            bias=QCOL[:, s:s + 1], scale=1.0,
                            accum_out=ACCA[:, 2 * ia + (0 if done == 0 else 1)
                                           :2 * ia + (1 if done == 0 else 2)])
                        done += wp
                # STT route (fused relu-sum on DVE)
                for iz, (_, s, c, w) in enumerate(STT_BLOCKS):
                    goff = (tiles[s] + 1) * 128 + c
                    done = 0
                    for (j, lo, wp) in tile_splits(goff, w):
                        nc.vector.scalar_tensor_tensor(
                            out=Z_tiles[iz][:, done:done + wp],
                            in0=QBT[j][:, lo:lo + wp],
                            scalar=QCOL[:, s:s + 1],
                            in1=ZERO[:, 0:wp], op0=OP.add, op1=OP.max,
                            accum_out=ACCA[:, 8 + 2 * iz + (0 if done == 0 else 1)
                                           :8 + 2 * iz + (1 if done == 0 else 2)])
                        done += wp

            # --- static PE stream: QBD then S tiles in PE_ORDER ---
            nc.tensor.matmul(PS[:], ONES[:], SD[:, 0:512], start=True, stop=False)
            nc.tensor.matmul(PS[:], ONES[:], SD[:, 512:1024], start=False, stop=False)
            for rank, i in enumerate(PE_ORDER):
                w = PE_BLOCKS[i][3]
                c2 = 0
                while c2 < w:
                    wp = min(512, w - c2)
                    nc.tensor.matmul(PS[:, 0:wp], ONES[:],
                                     S_tiles[i][:, c2:c2 + wp],
                                     start=False,
                                     stop=(rank == len(PE_ORDER) - 1) and
                                          (c2 + wp >= w))
                    c2 += wp

            # --- final combine: OUTV[:,0] = PE total (all rows equal),
            #     OUTV[:,1] = per-partition ACT+STT accums ---
            nc.vector.tensor_reduce(out=OUTV[:, 0:1], in_=PS[:], axis=AX.X,
                                    op=OP.add)
            nc.vector.tensor_reduce(out=OUTV[:, 1:2], in_=ACCA[:], axis=AX.X,
                                    op=OP.add)
            nc.scalar.dma_start(out=outv_d[:], in_=OUTV[:])

    nc.finalize()
    return nc


def get_program():
    if "nc" not in _CACHE:
        _CACHE["nc"] = build_program()
    return _CACHE["nc"]


# ---------------------------------------------------------------------------
# Host side
# ---------------------------------------------------------------------------

def build_inputs(q):
    import ml_dtypes
    BF = ml_dtypes.bfloat16
    q = q.astype(np.float32)
    rhs = np.full(QB_W, np.float32(PAD_VAL), np.float32)
    rhs[0:N] = 2.0 - q
    rhs16 = rhs.astype(BF)
    qb_rows = {j: np.broadcast_to(rhs16[TILES[j][0]:TILES[j][0] + TILES[j][1]],
                                  (128, TILES[j][1])).copy()
               for j in range(3)}

    in_maps = []
    for k in range(8):
        tiles = core_tiles(k)
        qcol = np.zeros((128, 8), np.float32)
        qbd = np.full((128, 1024), PAD_VAL, np.float32)
        tri = np.triu(np.ones((128, 128), bool), k=1)
        for s, t in enumerate(tiles):
            qcol[:, s] = q[t * 128:(t + 1) * 128]
            blk = q[t * 128:(t + 1) * 128]
            m = (2.0 - blk[None, :]) + blk[:, None]
            qbd[:, s * 128:(s + 1) * 128] = np.where(tri, m, PAD_VAL)
        in_maps.append({
            "qb_t0": qb_rows[0], "qb_t1": qb_rows[1], "qb_t2": qb_rows[2],
            "qbd": qbd.astype(BF), "qcol": qcol,
        })
    return in_maps


def tie_correction(labels, q, order):
    ls = labels[order]
    corr = 0.0
    i = 0
    n = len(ls)
    while i < n:
        j = i + 1
        while j < n and ls[j] == ls[i]:
            j += 1
        if j - i > 1:
            for u in range(i, j):
                for v in range(u + 1, j):
                    corr += 2.0 - max(0.0, 2.0 + float(q[u]) - float(q[v]))
        i = j
    return corr


def emulate(in_maps):
    """Numpy emulation of the device program for offline validation."""
    import ml_dtypes
    BF = ml_dtypes.bfloat16
    total = 0.0
    for k in range(8):
        m = in_maps[k]
        qbt = [m["qb_t0"].astype(np.float32), m["qb_t1"].astype(np.float32),
               m["qb_t2"].astype(np.float32)]
        qbd = m["qbd"].astype(np.float32)
        qcol = m["qcol"]
        tiles = core_tiles(k)
        core = np.maximum(qbd, 0.0).astype(BF).astype(np.float64).sum()
        acc = 0.0
        for kind, s, c, w in BLOCKS:
            goff = (tiles[s] + 1) * 128 + c
            parts = []
            for (j, lo, wp) in tile_splits(goff, w):
                parts.append(qbt[j][:, lo:lo + wp])
            sl = np.concatenate(parts, axis=1)
            t = sl + qcol[:, s][:, None]
            r = np.maximum(t, 0.0)
            if kind == "PE":
                core += r.astype(BF).astype(np.float64).sum()
            else:
                acc += r.astype(np.float64).sum()
        total += core + acc
    return total


def run(inputs, trace=False):
    from concourse.bass_utils import run_bass_kernel_spmd

    preds = np.asarray(inputs["preds"], dtype=np.float32)
    labels = np.asarray(inputs["labels"], dtype=np.float32)
    order = np.argsort(labels, kind="stable")
    q = preds[order]

    nc = get_program()
    in_maps = build_inputs(q)
    res = run_bass_kernel_spmd(nc, in_maps, core_ids=list(range(8)), trace=trace)
    total = 0.0
    for c in range(8):
        ov = res.results[c]["outv"]
        total += float(ov[0, 0])                      # PE route total
        total += ov[:, 1].astype(np.float64).sum()    # ACT+STT per-partition
    total += tie_correction(labels, q, order)
    return np.float32(total), res


def kernel(**inputs):
    out, _ = run(inputs, trace=False)
    return out
